# revision 46
# baseline (speedup 1.0000x reference)
"""Trainium2 Bass kernel for nn_LoopWithIf.

The reference loop
    for i in range(32):
        b = 3*a; s = sum(b); a = a+b if s>0 else a-b
collapses algebraically: the gate's sign is deterministic after the first
iteration, and scaling by 4 / -2 is exact in fp32 (powers of two), so
    out = inp * 2**64      if sum(inp) > 0
    out = inp * -(2**63)   otherwise

Kernel structure (single NEFF, SPMD over 8 NeuronCores, ~17MB/core kept
SBUF-resident so the data is read from HBM exactly once):
  phase 1   pipelined 2MB DMA loads + per-chunk reduce_sum on DVE (the
            last chunk is split in half to shorten the reduce tail)
  gate      direct SBUF->SBUF exchange of the [128,1] per-partition
            partials via remote_dma_broadcast (SWDGE), replacing the
            ncfw collective_compute AllGather (which costs ~40us of
            control-plane latency for a 512B payload).  SPMD-symmetric
            XOR addressing: broadcast k targets relative dest
            (drid=0, dtpb=k), i.e. physical tpb my_tpb^k, and writes
            column k of the receiver's gather buffer.  Receiver r's
            column k thus holds core (r^k)'s partial -- all 8 partials
            arrive, permuted, and only their SUM matters.  Desc-gen runs
            at kernel start (off critical path); one trigger_dma fires
            after the local combine; consumers wait remote_sem >= 14
            (7 peers x 2 lane-increments).
  factor    reduce the [128,8] gather buffer, broadcast the global total
            to all partitions with a single [128,128]-ones matmul, then
            two DVE tensor_scalar ops select 2**64 / -(2**63)
  phase 2   in-place scale by the factor (DVE, exact power-of-two
            multiply) + pipelined stores on the same HW DMA ring

Runtime branching (tc.If / value_load) crashes or fails codegen under
this PJRT/axon execution path, so the kernel is straight-line; the
factor select is pure data flow.
"""

import numpy as np

N_CORES = 8
ROWS = 32            # inp.shape[0]
ROWS_PER_CORE = ROWS // N_CORES
P = 128              # SBUF partitions

# per-core shard: 4*1024*1024 elements as [NCHUNK, P, F], chunk-contiguous
NCHUNK = 16
F = (ROWS_PER_CORE * 1024 * 1024) // (NCHUNK * P)   # 2048

_nc = None  # compiled kernel cache


def _build(nchunk=NCHUNK, p=P, f=F, n_cores=N_CORES):
    import concourse.bass as bass  # noqa: F401
    import concourse.bacc as bacc
    import concourse.mybir as mybir
    import concourse.tile as tile
    from concourse.instruction_name_ordered_set import InstructionNameOrderedSet

    f32 = mybir.dt.float32
    nc = bacc.Bacc(
        "TRN2",
        target_bir_lowering=False,
        debug=False,
        enable_asserts=False,
        num_devices=n_cores,
        num_swdge_queues=4,
    )
    inp_d = nc.dram_tensor("inp", [nchunk, p, f], f32, kind="ExternalInput").ap()
    out_d = nc.dram_tensor("out", [nchunk, p, f], f32, kind="ExternalOutput").ap()

    rsem = nc.alloc_semaphore("rdma_rsem")   # bumped by incoming peer DMAs
    lsem = nc.alloc_semaphore("rdma_lsem")   # bumped when our sends drain
    csem = nc.alloc_semaphore("combine_sem")  # stand-in: combine completion

    with tile.TileContext(nc) as tc:
        with (
            tc.tile_pool(name="data", bufs=1) as data_pool,
            tc.tile_pool(name="small", bufs=1) as small_pool,
            tc.tile_pool(name="psum", bufs=1, space="PSUM") as psum_pool,
            tc.tile_pool(name="dram", bufs=1, space="DRAM") as dram_pool,
        ):
            # Fire-and-forget 1-byte AllGather (emitted AFTER the exchange
            # trigger, below).  Nobody consumes the result; its presence
            # makes the runtime launch the 8 cores as one synchronized SPMD
            # program.  Without a real multi-core collective in the NEFF the
            # cores start ~ms apart (verified: singleton groups don't work
            # either) and every cross-core exchange pays the full stagger.
            # It must fire after our remote sends because active ncfw work
            # starves the SWDGE remote-DMA path until the collective
            # completes (verified: rsem arrivals land ~1.5us before AG end
            # when the doorbell rings first).
            u8 = mybir.dt.uint8
            sync_in = dram_pool.tile([1, 1], u8, name="launch_sync_in")
            sync_out = dram_pool.tile(
                [n_cores, 1], u8, name="launch_sync_out", addr_space="Shared"
            )
            chunks = [
                data_pool.tile([p, f], f32, name=f"xchunk{i}", tag=f"xchunk{i}")
                for i in range(nchunk)
            ]
            # one partials column per reduce piece.  The first chunk is
            # halved so DVE starts reducing ~6us earlier (it otherwise idles
            # until the first full chunk lands); the last chunk tapers so the
            # final reduce after the last byte lands is tiny.
            splits = [[1024, 1024]] + [[f]] * (nchunk - 2) + [[1024, 512, 256, 256]]
            assert all(sum(s) == f for s in splits)
            n_pieces = sum(len(s) for s in splits)
            partials = small_pool.tile([p, n_pieces], f32, name="partials")
            # gather buffer: col 0 = own partial (written by the local
            # combine), cols 1..7 = peers' partials (written by remote DMA)
            rbuf = small_pool.tile([p, n_cores], f32, name="rbuf")
            ones = small_pool.tile([p, p], f32, name="ones")
            nc.vector.memset(ones[:], 1.0)

            # phase 1: pipelined load + per-piece reduce
            col = 0
            for i in range(nchunk):
                off = 0
                for w in splits[i]:
                    nc.sync.dma_start(
                        chunks[i][:, off : off + w],
                        inp_d[i][:, off : off + w],
                    )
                    nc.vector.reduce_sum(
                        partials[:, col : col + 1],
                        chunks[i][:, off : off + w],
                        axis=mybir.AxisListType.X,
                    )
                    off += w
                    col += 1

            # local combine -> rbuf col 0 (also the exchange's source)
            combine_bi = nc.vector.reduce_sum(
                rbuf[:, 0:1], partials[:], axis=mybir.AxisListType.X
            )

            # remote-exchange desc-gen.  Broadcast k has its single real dest
            # at slot k (slot bit2 == dtpb bit2, so D2D-capable lane placement
            # is satisfied by construction).  MUST be emitted after the
            # combine: the preps READ rbuf[:,0:1], and trace order decides
            # whether Tile sees combine->prep as RAW (trigger waits for the
            # combine -- correct) or prep->combine as WAR (combine waits for
            # the trigger -- ships garbage partials).  Tile defers the preps'
            # source read to the trigger, so desc-gen itself can still be
            # scheduled early, off the critical path.
            # 7 single-dest broadcasts (XOR all-to-all).  Broadcast k has its
            # real dest at slot k (slot bit2 == dtpb bit2, D2D placement ok).
            # Emitted after the combine so Tile sees combine->prep as RAW.
            prep_bis, wait_bis, thresholds = [], [], []
            for k in range(1, n_cores):
                rdests = [None] * n_cores
                rdests[k] = (0, k)
                prep_bis.append(
                    nc.gpsimd.remote_dma_broadcast(
                        rbuf[:, k : k + 1],  # out_ap on the receiver
                        rbuf[:, 0 : 1],      # in_ap: our combined partial
                        rsem,
                        lsem,
                        rdests=rdests,
                    )
                )
            trigger_bis = [nc.gpsimd.trigger_dma(count=None)]

            # launch-sync collective doorbell, pinned after the trigger
            cc_bi = nc.gpsimd.collective_compute(
                "AllGather",
                mybir.AluOpType.bypass,
                replica_groups=[list(range(n_cores))],
                ins=[sync_in.opt()],
                outs=[sync_out.opt()],
            )
            cc_deps = InstructionNameOrderedSet()
            for tb in trigger_bis:
                cc_deps.add(tb.ins.name)
            cc_bi.ins.add_nosync_dependencies_from(cc_deps)

            # stand-in wait (csem never bumped, >=0 always true in the sim);
            # patched to rsem >= 14 after scheduling.  Pinned after the
            # combine so the scheduler can't hoist it ahead of the reduces.
            wbi = nc.vector.wait_ge(csem, 0)
            deps = InstructionNameOrderedSet()
            deps.add(combine_bi.ins.name)
            wbi.ins.add_nosync_dependencies_from(deps)
            wait_bis.append(wbi)
            thresholds.append(2 * (n_cores - 1))

            g = small_pool.tile([p, 1], f32, name="gsum")
            gred = nc.vector.reduce_sum(g[:], rbuf[:], axis=mybir.AxisListType.X)
            g_deps = InstructionNameOrderedSet()
            g_deps.add(wbi.ins.name)
            gred.ins.add_nosync_dependencies_from(g_deps)

            # broadcast the global total to all partitions in one matmul:
            # tot[m,0] = sum_k ones[k,m] * g[k,0]
            tot = psum_pool.tile([p, 1], f32, name="tot")
            nc.tensor.matmul(tot[:], ones[:], g[:])

            # factor = 1[tot>0] * 3*2^63 - 2^63  ->  2^64 or -2^63 (exact)
            fac = small_pool.tile([p, 1], f32, name="fac")
            nc.vector.tensor_scalar(fac[:], tot[:], 0.0, None, mybir.AluOpType.is_gt)
            nc.vector.tensor_scalar(
                fac[:],
                fac[:],
                float(3 * 2**63),
                float(-(2**63)),
                mybir.AluOpType.mult,
                mybir.AluOpType.add,
            )

            # phase 2: in-place scale (DVE) + store.  The first chunk is
            # scaled+stored in halves so the first store descriptor issues
            # half a scale earlier.
            h = f // 2
            for i in range(nchunk):
                if i == 0:
                    for off in (0, h):
                        nc.vector.tensor_scalar_mul(
                            chunks[i][:, off : off + h],
                            chunks[i][:, off : off + h],
                            fac[:],
                        )
                        nc.sync.dma_start(
                            out_d[i][:, off : off + h], chunks[i][:, off : off + h]
                        )
                else:
                    nc.vector.tensor_scalar_mul(chunks[i][:], chunks[i][:], fac[:])
                    nc.sync.dma_start(out_d[i], chunks[i][:])

    # Post-scheduling patches.
    # (1) Install the true cross-core ordering edges: round r's accumulate
    #     must see its partner's payload, i.e. rsem >= 2*(r+1).  Replace the
    #     csem stand-in waits in place.
    for wbi, thr in zip(wait_bis, thresholds):
        si = wbi.ins.sync_info
        rwait = mybir.SyncWait(
            sync_type="semaphore",
            id=rsem.num,
            ant_name=rsem.name,
            wait_mode="sem-ge-imm",
            wait_value=thr,
            wait_reg=None,
        )
        new_waits = [rwait if w.ant_name == csem.name else w for w in si.on_wait]
        assert any(w.ant_name == rsem.name for w in new_waits), new_waits
        wbi.ins.sync_info = mybir.SyncInfo(
            on_wait=new_waits, on_update=list(si.on_update)
        )

    # (2) Desc-gen reads no tensor data (the source read happens at SWDGE
    #     drain, gated by the trigger), but Tile gives each prep the RAW
    #     wait on its source (combine / previous add), putting serial Q7
    #     desc-gen on the critical path.  Move each prep's wait onto its
    #     round's trigger.
    for pb, tb in zip(prep_bis, trigger_bis):
        psi = pb.ins.sync_info
        pw = list(psi.on_wait) if psi is not None else []
        tsi = tb.ins.sync_info
        t_waits = list(tsi.on_wait) if tsi is not None else []
        t_ups = list(tsi.on_update) if tsi is not None else []
        if len(pw) == 1 and not t_waits:
            tb.ins.sync_info = mybir.SyncInfo(on_wait=pw, on_update=t_ups)
            pb.ins.sync_info = mybir.SyncInfo(
                on_wait=[], on_update=list(psi.on_update)
            )

    nc.compile()
    return nc


def _run(in_maps, trace=False):
    from concourse.bass_utils import run_bass_kernel_spmd

    global _nc
    if _nc is None:
        _nc = _build()
    return run_bass_kernel_spmd(
        _nc, in_maps, core_ids=list(range(N_CORES)), trace=trace
    )


def _shard(inp):
    return [
        np.ascontiguousarray(
            inp[c * ROWS_PER_CORE : (c + 1) * ROWS_PER_CORE]
        ).reshape(NCHUNK, P, F)
        for c in range(N_CORES)
    ]


def _unshard(results):
    out = np.empty((ROWS, 1024, 1024), dtype=np.float32)
    for c in range(N_CORES):
        out[c * ROWS_PER_CORE : (c + 1) * ROWS_PER_CORE] = results[c]["out"].reshape(
            ROWS_PER_CORE, 1024, 1024
        )
    return out


def kernel(**inputs):
    inp = np.ascontiguousarray(np.asarray(inputs["inp"], dtype=np.float32))
    res = _run([{"inp": s} for s in _shard(inp)], trace=False)
    return _unshard(res.results)


def run_traced(inputs):
    """Like kernel() but with NTFF profiling; returns (out, exec_time_ns)."""
    inp = np.ascontiguousarray(np.asarray(inputs["inp"], dtype=np.float32))
    res = _run([{"inp": s} for s in _shard(inp)], trace=True)
    return _unshard(res.results), res.exec_time_ns


# revision 47
# speedup vs baseline: 1.0612x; 1.0612x over previous
"""Trainium2 Bass kernel for nn_LoopWithIf.

The reference loop
    for i in range(32):
        b = 3*a; s = sum(b); a = a+b if s>0 else a-b
collapses algebraically: the gate's sign is deterministic after the first
iteration, and scaling by 4 / -2 is exact in fp32 (powers of two), so
    out = inp * 2**64      if sum(inp) > 0
    out = inp * -(2**63)   otherwise

Kernel structure (single NEFF, SPMD over 8 NeuronCores, ~17MB/core kept
SBUF-resident so the data is read from HBM exactly once):
  phase 1   pipelined 2MB DMA loads + per-chunk reduce_sum on DVE (the
            last chunk is split in half to shorten the reduce tail)
  gate      direct SBUF->SBUF exchange of the [128,1] per-partition
            partials via remote_dma_broadcast (SWDGE), replacing the
            ncfw collective_compute AllGather (which costs ~40us of
            control-plane latency for a 512B payload).  SPMD-symmetric
            XOR addressing: broadcast k targets relative dest
            (drid=0, dtpb=k), i.e. physical tpb my_tpb^k, and writes
            column k of the receiver's gather buffer.  Receiver r's
            column k thus holds core (r^k)'s partial -- all 8 partials
            arrive, permuted, and only their SUM matters.  Desc-gen runs
            at kernel start (off critical path); one trigger_dma fires
            after the local combine; consumers wait remote_sem >= 14
            (7 peers x 2 lane-increments).
  factor    reduce the [128,8] gather buffer, broadcast the global total
            to all partitions with a single [128,128]-ones matmul, then
            two DVE tensor_scalar ops select 2**64 / -(2**63)
  phase 2   in-place scale by the factor (DVE, exact power-of-two
            multiply) + pipelined stores on the same HW DMA ring

Runtime branching (tc.If / value_load) crashes or fails codegen under
this PJRT/axon execution path, so the kernel is straight-line; the
factor select is pure data flow.
"""

import numpy as np

N_CORES = 8
ROWS = 32            # inp.shape[0]
ROWS_PER_CORE = ROWS // N_CORES
P = 128              # SBUF partitions

# per-core shard: 4*1024*1024 elements as [NCHUNK, P, F], chunk-contiguous
NCHUNK = 16
F = (ROWS_PER_CORE * 1024 * 1024) // (NCHUNK * P)   # 2048

_nc = None  # compiled kernel cache


def _build(nchunk=NCHUNK, p=P, f=F, n_cores=N_CORES):
    import concourse.bass as bass  # noqa: F401
    import concourse.bacc as bacc
    import concourse.mybir as mybir
    import concourse.tile as tile
    from concourse.instruction_name_ordered_set import InstructionNameOrderedSet

    f32 = mybir.dt.float32
    nc = bacc.Bacc(
        "TRN2",
        target_bir_lowering=False,
        debug=False,
        enable_asserts=False,
        num_devices=n_cores,
        num_swdge_queues=4,
    )
    inp_d = nc.dram_tensor("inp", [nchunk, p, f], f32, kind="ExternalInput").ap()
    out_d = nc.dram_tensor("out", [nchunk, p, f], f32, kind="ExternalOutput").ap()

    rsem = nc.alloc_semaphore("rdma_rsem")   # bumped by incoming peer DMAs
    lsem = nc.alloc_semaphore("rdma_lsem")   # bumped when our sends drain
    csem = nc.alloc_semaphore("combine_sem")  # stand-in: combine completion

    with tile.TileContext(nc) as tc:
        with (
            tc.tile_pool(name="data", bufs=1) as data_pool,
            tc.tile_pool(name="small", bufs=1) as small_pool,
            tc.tile_pool(name="psum", bufs=1, space="PSUM") as psum_pool,
            tc.tile_pool(name="dram", bufs=1, space="DRAM") as dram_pool,
        ):
            # Fire-and-forget 1-byte AllGather (emitted AFTER the exchange
            # trigger, below).  Nobody consumes the result; its presence
            # makes the runtime launch the 8 cores as one synchronized SPMD
            # program.  Without a real multi-core collective in the NEFF the
            # cores start ~ms apart (verified: singleton groups don't work
            # either) and every cross-core exchange pays the full stagger.
            # It must fire after our remote sends because active ncfw work
            # starves the SWDGE remote-DMA path until the collective
            # completes (verified: rsem arrivals land ~1.5us before AG end
            # when the doorbell rings first).
            u8 = mybir.dt.uint8
            sync_in = dram_pool.tile([1, 1], u8, name="launch_sync_in")
            sync_out = dram_pool.tile(
                [n_cores, 1], u8, name="launch_sync_out", addr_space="Shared"
            )
            chunks = [
                data_pool.tile([p, f], f32, name=f"xchunk{i}", tag=f"xchunk{i}")
                for i in range(nchunk)
            ]
            # one partials column per reduce; the last chunk is loaded+reduced
            # in decreasing pieces so the final reduce (what the trigger
            # waits on after the last byte lands) is tiny
            tail_splits = [1024, 512, 256, 256]
            assert sum(tail_splits) == f
            partials = small_pool.tile(
                [p, nchunk - 1 + len(tail_splits)], f32, name="partials"
            )
            # gather buffer: col 0 = own partial (written by the local
            # combine), cols 1..7 = peers' partials (written by remote DMA)
            rbuf = small_pool.tile([p, n_cores], f32, name="rbuf")
            ones = small_pool.tile([p, p], f32, name="ones")
            nc.vector.memset(ones[:], 1.0)

            # phase 1: pipelined load + per-chunk reduce
            for i in range(nchunk):
                if i < nchunk - 1:
                    nc.sync.dma_start(chunks[i][:], inp_d[i])
                    nc.vector.reduce_sum(
                        partials[:, i : i + 1], chunks[i][:], axis=mybir.AxisListType.X
                    )
                else:
                    off = 0
                    for j, w in enumerate(tail_splits):
                        nc.sync.dma_start(
                            chunks[i][:, off : off + w],
                            inp_d[i][:, off : off + w],
                        )
                        nc.vector.reduce_sum(
                            partials[:, i + j : i + j + 1],
                            chunks[i][:, off : off + w],
                            axis=mybir.AxisListType.X,
                        )
                        off += w

            # local combine -> rbuf col 0 (also the exchange's source)
            combine_bi = nc.vector.reduce_sum(
                rbuf[:, 0:1], partials[:], axis=mybir.AxisListType.X
            )

            # remote-exchange desc-gen.  Broadcast k has its single real dest
            # at slot k (slot bit2 == dtpb bit2, so D2D-capable lane placement
            # is satisfied by construction).  MUST be emitted after the
            # combine: the preps READ rbuf[:,0:1], and trace order decides
            # whether Tile sees combine->prep as RAW (trigger waits for the
            # combine -- correct) or prep->combine as WAR (combine waits for
            # the trigger -- ships garbage partials).  Tile defers the preps'
            # source read to the trigger, so desc-gen itself can still be
            # scheduled early, off the critical path.
            # 7 single-dest broadcasts (XOR all-to-all).  Broadcast k has its
            # real dest at slot k (slot bit2 == dtpb bit2, D2D placement ok).
            # Emitted after the combine so Tile sees combine->prep as RAW.
            prep_bis, wait_bis, thresholds = [], [], []
            for k in range(1, n_cores):
                rdests = [None] * n_cores
                rdests[k] = (0, k)
                prep_bis.append(
                    nc.gpsimd.remote_dma_broadcast(
                        rbuf[:, k : k + 1],  # out_ap on the receiver
                        rbuf[:, 0 : 1],      # in_ap: our combined partial
                        rsem,
                        lsem,
                        rdests=rdests,
                    )
                )
            trigger_bis = [nc.gpsimd.trigger_dma(count=None)]

            # launch-sync collective doorbell, pinned after the trigger
            cc_bi = nc.gpsimd.collective_compute(
                "AllGather",
                mybir.AluOpType.bypass,
                replica_groups=[list(range(n_cores))],
                ins=[sync_in.opt()],
                outs=[sync_out.opt()],
            )
            cc_deps = InstructionNameOrderedSet()
            for tb in trigger_bis:
                cc_deps.add(tb.ins.name)
            cc_bi.ins.add_nosync_dependencies_from(cc_deps)

            # stand-in wait (csem never bumped, >=0 always true in the sim);
            # patched to rsem >= 14 after scheduling.  Pinned after the
            # combine so the scheduler can't hoist it ahead of the reduces.
            wbi = nc.vector.wait_ge(csem, 0)
            deps = InstructionNameOrderedSet()
            deps.add(combine_bi.ins.name)
            wbi.ins.add_nosync_dependencies_from(deps)
            wait_bis.append(wbi)
            thresholds.append(2 * (n_cores - 1))

            g = small_pool.tile([p, 1], f32, name="gsum")
            gred = nc.vector.reduce_sum(g[:], rbuf[:], axis=mybir.AxisListType.X)
            g_deps = InstructionNameOrderedSet()
            g_deps.add(wbi.ins.name)
            gred.ins.add_nosync_dependencies_from(g_deps)

            # broadcast the global total to all partitions in one matmul:
            # tot[m,0] = sum_k ones[k,m] * g[k,0]
            tot = psum_pool.tile([p, 1], f32, name="tot")
            nc.tensor.matmul(tot[:], ones[:], g[:])

            # factor = 1[tot>0] * 3*2^63 - 2^63  ->  2^64 or -2^63 (exact)
            fac = small_pool.tile([p, 1], f32, name="fac")
            nc.vector.tensor_scalar(fac[:], tot[:], 0.0, None, mybir.AluOpType.is_gt)
            nc.vector.tensor_scalar(
                fac[:],
                fac[:],
                float(3 * 2**63),
                float(-(2**63)),
                mybir.AluOpType.mult,
                mybir.AluOpType.add,
            )

            # phase 2: in-place scale (DVE) + store
            for i in range(nchunk):
                nc.vector.tensor_scalar_mul(chunks[i][:], chunks[i][:], fac[:])
                nc.sync.dma_start(out_d[i], chunks[i][:])

    # Post-scheduling patches.
    # (1) Install the true cross-core ordering edges: round r's accumulate
    #     must see its partner's payload, i.e. rsem >= 2*(r+1).  Replace the
    #     csem stand-in waits in place.
    for wbi, thr in zip(wait_bis, thresholds):
        si = wbi.ins.sync_info
        rwait = mybir.SyncWait(
            sync_type="semaphore",
            id=rsem.num,
            ant_name=rsem.name,
            wait_mode="sem-ge-imm",
            wait_value=thr,
            wait_reg=None,
        )
        new_waits = [rwait if w.ant_name == csem.name else w for w in si.on_wait]
        assert any(w.ant_name == rsem.name for w in new_waits), new_waits
        wbi.ins.sync_info = mybir.SyncInfo(
            on_wait=new_waits, on_update=list(si.on_update)
        )

    # (2) Desc-gen reads no tensor data (the source read happens at SWDGE
    #     drain, gated by the trigger), but Tile gives each prep the RAW
    #     wait on its source (combine / previous add), putting serial Q7
    #     desc-gen on the critical path.  Move each prep's wait onto its
    #     round's trigger.
    for pb, tb in zip(prep_bis, trigger_bis):
        psi = pb.ins.sync_info
        pw = list(psi.on_wait) if psi is not None else []
        tsi = tb.ins.sync_info
        t_waits = list(tsi.on_wait) if tsi is not None else []
        t_ups = list(tsi.on_update) if tsi is not None else []
        if len(pw) == 1 and not t_waits:
            tb.ins.sync_info = mybir.SyncInfo(on_wait=pw, on_update=t_ups)
            pb.ins.sync_info = mybir.SyncInfo(
                on_wait=[], on_update=list(psi.on_update)
            )

    nc.compile()
    return nc


def _run(in_maps, trace=False):
    from concourse.bass_utils import run_bass_kernel_spmd

    global _nc
    if _nc is None:
        _nc = _build()
    return run_bass_kernel_spmd(
        _nc, in_maps, core_ids=list(range(N_CORES)), trace=trace
    )


def _shard(inp):
    return [
        np.ascontiguousarray(
            inp[c * ROWS_PER_CORE : (c + 1) * ROWS_PER_CORE]
        ).reshape(NCHUNK, P, F)
        for c in range(N_CORES)
    ]


def _unshard(results):
    out = np.empty((ROWS, 1024, 1024), dtype=np.float32)
    for c in range(N_CORES):
        out[c * ROWS_PER_CORE : (c + 1) * ROWS_PER_CORE] = results[c]["out"].reshape(
            ROWS_PER_CORE, 1024, 1024
        )
    return out


def kernel(**inputs):
    inp = np.ascontiguousarray(np.asarray(inputs["inp"], dtype=np.float32))
    res = _run([{"inp": s} for s in _shard(inp)], trace=False)
    return _unshard(res.results)


def run_traced(inputs):
    """Like kernel() but with NTFF profiling; returns (out, exec_time_ns)."""
    inp = np.ascontiguousarray(np.asarray(inputs["inp"], dtype=np.float32))
    res = _run([{"inp": s} for s in _shard(inp)], trace=True)
    return _unshard(res.results), res.exec_time_ns


# revision 48
# speedup vs baseline: 1.2346x; 1.1634x over previous
"""Trainium2 Bass kernel for nn_LoopWithIf.

The reference loop
    for i in range(32):
        b = 3*a; s = sum(b); a = a+b if s>0 else a-b
collapses algebraically: the gate's sign is deterministic after the first
iteration, and scaling by 4 / -2 is exact in fp32 (powers of two), so
    out = inp * 2**64      if sum(inp) > 0
    out = inp * -(2**63)   otherwise

Kernel structure (single NEFF, SPMD over 8 NeuronCores, ~17MB/core kept
SBUF-resident so the data is read from HBM exactly once):
  phase 1   pipelined 1MB DMA loads + per-chunk reduce_sum on DVE (the
            last chunk tapers 0.5/0.25/0.125/0.125MB so the reduce tail
            after the final byte lands is tiny)
  gate      direct SBUF->SBUF exchange of the [128,1] per-partition
            partials via remote_dma_broadcast (SWDGE), replacing the
            ncfw collective_compute AllGather (which costs ~40us of
            control-plane latency for a 512B payload).  SPMD-symmetric
            XOR addressing: broadcast k targets relative dest
            (drid=0, dtpb=k), i.e. physical tpb my_tpb^k, and writes
            column k of the receiver's gather buffer.  Receiver r's
            column k thus holds core (r^k)'s partial -- all 8 partials
            arrive, permuted, and only their SUM matters.  Desc-gen runs
            at kernel start (off critical path); one trigger_dma fires
            after the local combine; consumers wait remote_sem >= 14
            (7 peers x 2 lane-increments).
  factor    reduce the [128,8] gather buffer, broadcast the global total
            to all partitions with a single [128,128]-ones matmul, then
            two DVE tensor_scalar ops select 2**64 / -(2**63)
  phase 2   in-place scale by the factor (DVE, exact power-of-two
            multiply) + pipelined stores on the same HW DMA ring

Runtime branching (tc.If / value_load) crashes or fails codegen under
this PJRT/axon execution path, so the kernel is straight-line; the
factor select is pure data flow.
"""

import numpy as np

N_CORES = 8
ROWS = 32            # inp.shape[0]
ROWS_PER_CORE = ROWS // N_CORES
P = 128              # SBUF partitions

# per-core shard: 4*1024*1024 elements as [NCHUNK, P, F], chunk-contiguous
NCHUNK = 16
F = (ROWS_PER_CORE * 1024 * 1024) // (NCHUNK * P)   # 2048

_nc = None  # compiled kernel cache


def _build(nchunk=NCHUNK, p=P, f=F, n_cores=N_CORES):
    import concourse.bass as bass  # noqa: F401
    import concourse.bacc as bacc
    import concourse.mybir as mybir
    import concourse.tile as tile
    from concourse.instruction_name_ordered_set import InstructionNameOrderedSet

    f32 = mybir.dt.float32
    nc = bacc.Bacc(
        "TRN2",
        target_bir_lowering=False,
        debug=False,
        enable_asserts=False,
        num_devices=n_cores,
        num_swdge_queues=4,
    )
    inp_d = nc.dram_tensor("inp", [nchunk, p, f], f32, kind="ExternalInput").ap()
    out_d = nc.dram_tensor("out", [nchunk, p, f], f32, kind="ExternalOutput").ap()

    rsem = nc.alloc_semaphore("rdma_rsem")   # bumped by incoming peer DMAs
    lsem = nc.alloc_semaphore("rdma_lsem")   # bumped when our sends drain
    csem = nc.alloc_semaphore("combine_sem")  # stand-in: combine completion

    with tile.TileContext(nc) as tc:
        with (
            tc.tile_pool(name="data", bufs=1) as data_pool,
            tc.tile_pool(name="small", bufs=1) as small_pool,
            tc.tile_pool(name="psum", bufs=1, space="PSUM") as psum_pool,
            tc.tile_pool(name="dram", bufs=1, space="DRAM") as dram_pool,
        ):
            # Fire-and-forget 1-byte AllGather (emitted AFTER the exchange
            # trigger, below).  Nobody consumes the result; its presence
            # makes the runtime launch the 8 cores as one synchronized SPMD
            # program.  Without a real multi-core collective in the NEFF the
            # cores start ~ms apart (verified: singleton groups don't work
            # either) and every cross-core exchange pays the full stagger.
            # It must fire after our remote sends because active ncfw work
            # starves the SWDGE remote-DMA path until the collective
            # completes (verified: rsem arrivals land ~1.5us before AG end
            # when the doorbell rings first).
            u8 = mybir.dt.uint8
            sync_in = dram_pool.tile([1, 1], u8, name="launch_sync_in")
            sync_out = dram_pool.tile(
                [n_cores, 1], u8, name="launch_sync_out", addr_space="Shared"
            )
            chunks = [
                data_pool.tile([p, f], f32, name=f"xchunk{i}", tag=f"xchunk{i}")
                for i in range(nchunk)
            ]
            # one partials column per reduce; the last chunk is loaded+reduced
            # in decreasing pieces so the final reduce (what the trigger
            # waits on after the last byte lands) is tiny
            tail_splits = [1024, 512, 256, 256]
            assert sum(tail_splits) == f
            partials = small_pool.tile(
                [p, nchunk - 1 + len(tail_splits)], f32, name="partials"
            )
            # gather buffer: col 0 = own partial (written by the local
            # combine), cols 1..7 = peers' partials (written by remote DMA)
            rbuf = small_pool.tile([p, n_cores], f32, name="rbuf")
            ones = small_pool.tile([p, p], f32, name="ones")
            nc.vector.memset(ones[:], 1.0)

            # phase 1: pipelined load + per-chunk reduce
            for i in range(nchunk):
                if i < nchunk - 1:
                    nc.sync.dma_start(chunks[i][:], inp_d[i])
                    nc.vector.reduce_sum(
                        partials[:, i : i + 1], chunks[i][:], axis=mybir.AxisListType.X
                    )
                else:
                    off = 0
                    for j, w in enumerate(tail_splits):
                        nc.sync.dma_start(
                            chunks[i][:, off : off + w],
                            inp_d[i][:, off : off + w],
                        )
                        nc.vector.reduce_sum(
                            partials[:, i + j : i + j + 1],
                            chunks[i][:, off : off + w],
                            axis=mybir.AxisListType.X,
                        )
                        off += w

            # local combine -> rbuf col 0 (also the exchange's source)
            combine_bi = nc.vector.reduce_sum(
                rbuf[:, 0:1], partials[:], axis=mybir.AxisListType.X
            )

            # remote-exchange desc-gen.  Broadcast k has its single real dest
            # at slot k (slot bit2 == dtpb bit2, so D2D-capable lane placement
            # is satisfied by construction).  MUST be emitted after the
            # combine: the preps READ rbuf[:,0:1], and trace order decides
            # whether Tile sees combine->prep as RAW (trigger waits for the
            # combine -- correct) or prep->combine as WAR (combine waits for
            # the trigger -- ships garbage partials).  Tile defers the preps'
            # source read to the trigger, so desc-gen itself can still be
            # scheduled early, off the critical path.
            # 7 single-dest broadcasts (XOR all-to-all).  Broadcast k has its
            # real dest at slot k (slot bit2 == dtpb bit2, D2D placement ok).
            # Emitted after the combine so Tile sees combine->prep as RAW.
            prep_bis, wait_bis, thresholds = [], [], []
            for k in range(1, n_cores):
                rdests = [None] * n_cores
                rdests[k] = (0, k)
                prep_bis.append(
                    nc.gpsimd.remote_dma_broadcast(
                        rbuf[:, k : k + 1],  # out_ap on the receiver
                        rbuf[:, 0 : 1],      # in_ap: our combined partial
                        rsem,
                        lsem,
                        rdests=rdests,
                    )
                )
            trigger_bis = [nc.gpsimd.trigger_dma(count=None)]

            # launch-sync collective doorbell, pinned after the trigger
            cc_bi = nc.gpsimd.collective_compute(
                "AllGather",
                mybir.AluOpType.bypass,
                replica_groups=[list(range(n_cores))],
                ins=[sync_in.opt()],
                outs=[sync_out.opt()],
            )
            cc_deps = InstructionNameOrderedSet()
            for tb in trigger_bis:
                cc_deps.add(tb.ins.name)
            cc_bi.ins.add_nosync_dependencies_from(cc_deps)

            # stand-in wait (csem never bumped, >=0 always true in the sim);
            # patched to rsem >= 14 after scheduling.  Pinned after the
            # combine so the scheduler can't hoist it ahead of the reduces.
            wbi = nc.vector.wait_ge(csem, 0)
            deps = InstructionNameOrderedSet()
            deps.add(combine_bi.ins.name)
            wbi.ins.add_nosync_dependencies_from(deps)
            wait_bis.append(wbi)
            thresholds.append(2 * (n_cores - 1))

            g = small_pool.tile([p, 1], f32, name="gsum")
            gred = nc.vector.reduce_sum(g[:], rbuf[:], axis=mybir.AxisListType.X)
            g_deps = InstructionNameOrderedSet()
            g_deps.add(wbi.ins.name)
            gred.ins.add_nosync_dependencies_from(g_deps)

            # broadcast the global total to all partitions in one matmul:
            # tot[m,0] = sum_k ones[k,m] * g[k,0]
            tot = psum_pool.tile([p, 1], f32, name="tot")
            nc.tensor.matmul(tot[:], ones[:], g[:])

            # factor = 1[tot>0] * 3*2^63 - 2^63  ->  2^64 or -2^63 (exact)
            fac = small_pool.tile([p, 1], f32, name="fac")
            nc.vector.tensor_scalar(fac[:], tot[:], 0.0, None, mybir.AluOpType.is_gt)
            nc.vector.tensor_scalar(
                fac[:],
                fac[:],
                float(3 * 2**63),
                float(-(2**63)),
                mybir.AluOpType.mult,
                mybir.AluOpType.add,
            )

            # phase 2: in-place scale (DVE) + store
            for i in range(nchunk):
                nc.vector.tensor_scalar_mul(chunks[i][:], chunks[i][:], fac[:])
                nc.sync.dma_start(out_d[i], chunks[i][:])

    # Post-scheduling patches.
    # (1) Install the true cross-core ordering edges: round r's accumulate
    #     must see its partner's payload, i.e. rsem >= 2*(r+1).  Replace the
    #     csem stand-in waits in place.
    for wbi, thr in zip(wait_bis, thresholds):
        si = wbi.ins.sync_info
        rwait = mybir.SyncWait(
            sync_type="semaphore",
            id=rsem.num,
            ant_name=rsem.name,
            wait_mode="sem-ge-imm",
            wait_value=thr,
            wait_reg=None,
        )
        new_waits = [rwait if w.ant_name == csem.name else w for w in si.on_wait]
        assert any(w.ant_name == rsem.name for w in new_waits), new_waits
        wbi.ins.sync_info = mybir.SyncInfo(
            on_wait=new_waits, on_update=list(si.on_update)
        )

    # (2) Desc-gen reads no tensor data (the source read happens at SWDGE
    #     drain, gated by the trigger), but Tile gives each prep the RAW
    #     wait on its source (combine / previous add), putting serial Q7
    #     desc-gen on the critical path.  Move each prep's wait onto its
    #     round's trigger.
    for pb, tb in zip(prep_bis, trigger_bis):
        psi = pb.ins.sync_info
        pw = list(psi.on_wait) if psi is not None else []
        tsi = tb.ins.sync_info
        t_waits = list(tsi.on_wait) if tsi is not None else []
        t_ups = list(tsi.on_update) if tsi is not None else []
        if len(pw) == 1 and not t_waits:
            tb.ins.sync_info = mybir.SyncInfo(on_wait=pw, on_update=t_ups)
            pb.ins.sync_info = mybir.SyncInfo(
                on_wait=[], on_update=list(psi.on_update)
            )

    nc.compile()
    return nc


def _run(in_maps, trace=False):
    from concourse.bass_utils import run_bass_kernel_spmd

    global _nc
    if _nc is None:
        _nc = _build()
    return run_bass_kernel_spmd(
        _nc, in_maps, core_ids=list(range(N_CORES)), trace=trace
    )


def _shard(inp):
    return [
        np.ascontiguousarray(
            inp[c * ROWS_PER_CORE : (c + 1) * ROWS_PER_CORE]
        ).reshape(NCHUNK, P, F)
        for c in range(N_CORES)
    ]


def _unshard(results):
    out = np.empty((ROWS, 1024, 1024), dtype=np.float32)
    for c in range(N_CORES):
        out[c * ROWS_PER_CORE : (c + 1) * ROWS_PER_CORE] = results[c]["out"].reshape(
            ROWS_PER_CORE, 1024, 1024
        )
    return out


def kernel(**inputs):
    inp = np.ascontiguousarray(np.asarray(inputs["inp"], dtype=np.float32))
    res = _run([{"inp": s} for s in _shard(inp)], trace=False)
    return _unshard(res.results)


def run_traced(inputs):
    """Like kernel() but with NTFF profiling; returns (out, exec_time_ns)."""
    inp = np.ascontiguousarray(np.asarray(inputs["inp"], dtype=np.float32))
    res = _run([{"inp": s} for s in _shard(inp)], trace=True)
    return _unshard(res.results), res.exec_time_ns
